# revision 32
# baseline (speedup 1.0000x reference)
"""Multi-head attention (B=8, S=1024, E=768, H=12) on 8 trn2 NeuronCores.

Strategy: batch-parallel — core b processes batch element b end-to-end, no
collectives.  Projections/attention/output in bf16 with fp32 PSUM; the
score matmul runs in fp8-e4m3 DoubleRow mode (2 contraction rows per
partition -> full 128-row array use at half the stream cycles) with a
q-side residual (q ~= q8 + qr8) so only the k-side fp8 quantization error
survives (~6e-3 end-to-end, vs 2.4e-3 all-bf16).

Per-core dataflow (token s/t, feature e, head h, head-dim d):
  xT[e, s]    PE-transpose of x in bf16 (cast on ACT first; 2 DMA waves)
  q/k proj    psum[hd, s-chunk] = Wq/Wk-tile^T @ xT; DVE writes
              qf[128, 2, 1024] fp8 (blk0 = fp8(psum+b), blk1 = residual),
              kf same with blk1 = copy of blk0 (SBUF DMA)
  v[t, hdA]   xT_aug^T @ WvT_aug  (ones row gives bias; col h*65+64 is
              all-ones -> softmax denominator), interleaved with pair 0
  scoresT     DoubleRow fp8: lhsT = kf[h*64:, :, t-tile], rhs = qf[h*64:,
              :, s-chunk]; 256 cycles per [128, 512] output
  expT        ACT exp with scale=1/8 (scores kept unscaled), bf16
  attn_aug    v_aug^T @ expT accumulated over t (row 64 = denominator)
  normalize   DVE reciprocal of den row; Pool partition_broadcast to 64
              rows; Pool/DVE multiply -> catT bf16
  out[s, f]   catT_aug^T @ WoT_aug (bias row folded), ACT copy, DMA out
"""

import os
import numpy as np
import ml_dtypes

B, S, E, H, DH = 8, 1024, 768, 12, 64
EA = E + 1          # augmented contraction dim (ones/bias row)
HW = DH + 1         # per-head V width (d cols + ones col)
VW = H * HW         # 780
NT = S // 128       # 8 token tiles
NE = E // 128       # 6 feature tiles

_cache = {}


def _split_multiwaits(nc):
    """This toolchain's walrus encodes at most one sync-wait per instruction
    (two for EventSemaphore).  Tile's epilogue can attach more; hoist the
    extras onto same-engine NOPs placed immediately before the instruction —
    the engine sequencer executes in order, so semantics are unchanged."""
    import concourse.mybir as mybir

    for bb in nc.main_func.blocks:
        out, changed = [], False
        for ins in bb.instructions:
            si = ins.sync_info
            cap = 2 if isinstance(ins, mybir.InstEventSemaphore) else 1
            if si is not None and si.on_wait and len(si.on_wait) > cap:
                waits = list(si.on_wait)
                for w_i, w in enumerate(waits[:-cap]):
                    out.append(mybir.InstNoOp(
                        name=f"{ins.name}-wsplit{w_i}",
                        engine=ins.engine,
                        sync_info=mybir.SyncInfo(on_wait=[w], on_update=[]),
                        bass_nofuse=True,
                    ))
                ins.sync_info = mybir.SyncInfo(
                    on_wait=waits[-cap:], on_update=list(si.on_update))
                changed = True
            out.append(ins)
        if changed:
            bb.instructions = out


def _dedupe_ldweights(nc):
    """Delete an InstLdweights when the immediately-preceding PE-stream
    instructions are its identical twin followed only by plain (non-transpose)
    matmuls — the weights are still resident in the array.  Only waitless,
    updateless LDWs are removed."""
    import concourse.mybir as mybir

    ndel = 0
    for bb in nc.main_func.blocks:
        out = []
        prev_key = None          # signature of weights currently in the array
        changed = False
        for ins in bb.instructions:
            if isinstance(ins, mybir.InstLdweights):
                si = ins.sync_info
                clean = not si or (not si.on_wait and not si.on_update)
                key = (str(ins.ins[0]), str(ins.tile_position),
                       str(ins.perf_mode), str(ins.is_transpose))
                if clean and key == prev_key:
                    ndel += 1
                    changed = True
                    continue
                prev_key = key
            elif isinstance(ins, mybir.InstMatmult):
                if ins.is_transpose:
                    prev_key = None   # transpose streams data into the array
            elif ins.engine == mybir.EngineType.PE:
                prev_key = None
            out.append(ins)
        if changed:
            bb.instructions = out
    return ndel


def _build_bass(split_waits=True):
    import concourse.bass as bass
    import concourse.tile as tile
    import concourse.mybir as mybir
    from contextlib import ExitStack

    f32 = mybir.dt.float32
    bf16 = mybir.dt.bfloat16
    EXP = mybir.ActivationFunctionType.Exp

    nc = bass.Bass(trn_type="TRN2")

    # All inputs arrive host-pre-tiled as the exact SBUF image
    # [128, NE*width] so every load is 128 contiguous descriptors.
    xt_d = nc.dram_tensor("xtr", [128, NE * S], bf16, kind="ExternalInput")
    wqt_d = nc.dram_tensor("wqt", [128, NE * E], bf16, kind="ExternalInput")
    wkt_d = nc.dram_tensor("wkt", [128, NE * E], bf16, kind="ExternalInput")
    bq_d = nc.dram_tensor("bq", [128, NE], f32, kind="ExternalInput")
    bk_d = nc.dram_tensor("bk", [128, NE], f32, kind="ExternalInput")
    wvt_d = nc.dram_tensor("wvt", [128, NE * VW], bf16, kind="ExternalInput")
    wvb_d = nc.dram_tensor("wvb", [1, VW], bf16, kind="ExternalInput")
    wot_d = nc.dram_tensor("wot", [128, NE * E], bf16, kind="ExternalInput")
    wob_d = nc.dram_tensor("wob", [1, E], bf16, kind="ExternalInput")
    out_d = nc.dram_tensor("out", [S, E], f32, kind="ExternalOutput")

    with tile.TileContext(nc) as tc, ExitStack() as ctx:
        singles = ctx.enter_context(tc.tile_pool(name="singles", bufs=1))

        ones_row = singles.tile([1, 1024], bf16)
        nc.vector.memset(ones_row, 1.0)

        # ---- input DMAs.  Big transfers serialize on the SP queue in
        # priority order (descriptor-gen of DMA n+1 overlaps transfer n);
        # the two tiny bias loads go to the Pool queue. ----
        xt_all = singles.tile([128, NE * S], bf16, tag="xta", name="xtall")
        nc.sync.dma_start(out=xt_all, in_=xt_d[:, :])
        xt = [xt_all[:, j * S:(j + 1) * S] for j in range(NE)]

        # biases as one [128, NE] tile each (column m = k-tile m)
        bq_sb = singles.tile([128, NE], f32, tag="bqs", name="bqs")
        nc.gpsimd.dma_start(out=bq_sb, in_=bq_d[:, :])
        bk_sb = singles.tile([128, NE], f32, tag="bks", name="bks")
        nc.gpsimd.dma_start(out=bk_sb, in_=bk_d[:, :])
        bqs = [bq_sb[:, m:m + 1] for m in range(NE)]
        bks = [bk_sb[:, m:m + 1] for m in range(NE)]

        class WView:
            """All k-tiles of a weight in one SBUF tile (one DMA)."""
            def __init__(self, all_tile, width, bias_tile):
                self.all, self.width, self.bias = all_tile, width, bias_tile

            def __getitem__(self, k):
                if self.bias is not None and k == NE:
                    return self.bias
                return _WSlice(self, k)

        class _WSlice:
            def __init__(self, v, k):
                self.v, self.k = v, k

            def __getitem__(self, idx):
                _, cols = idx
                off = self.k * self.v.width
                return self.v.all[:, off + cols.start:off + cols.stop]

        def load_w(dram, width, bias_dram):
            t = singles.tile([128, NE * width], bf16, tag=f"w{dram.name}",
                             name=f"w{dram.name}")
            nc.sync.dma_start(out=t, in_=dram[:, :])
            bias_t = None
            if bias_dram is not None:
                bias_t = singles.tile([1, width], bf16, tag=f"w{dram.name}b",
                                      name=f"w{dram.name}b")
                nc.gpsimd.dma_start(out=bias_t, in_=bias_dram[:, :])
            return WView(t, width, bias_t)

        wq = load_w(wqt_d, E, None)
        wk = load_w(wkt_d, E, None)
        wv = load_w(wvt_d, VW, wvb_d)
        wo = load_w(wot_d, E, wob_d)

        def xa(k):  # augmented xT rows
            return xt[k] if k < NE else ones_row

        # ---- steady-state tiles ----
        vt = [singles.tile([128, VW], bf16, tag=f"vt{i}", name=f"vt{i}")
              for i in range(NT)]
        catt = [singles.tile([128, S], bf16, tag=f"ct{j}", name=f"ct{j}")
                for j in range(NE)]

        with ExitStack() as sctx:
            qk8p = sctx.enter_context(tc.tile_pool(name="qk8", bufs=2))
            expp = sctx.enter_context(tc.tile_pool(name="exp", bufs=18))
            normp = sctx.enter_context(tc.tile_pool(name="norm", bufs=2))
            ps_proj = sctx.enter_context(
                tc.tile_pool(name="ps_proj", bufs=2, space="PSUM"))
            ps_sc = sctx.enter_context(
                tc.tile_pool(name="ps_sc", bufs=2, space="PSUM"))
            dscr = sctx.enter_context(
                tc.tile_pool(name="dscr", bufs=8, space="DRAM"))

            qts, kts = {}, {}

            def emit_qk(hp):
                qt = qk8p.tile([128, S], bf16, tag="qt", name=f"qt{hp}")
                kt = qk8p.tile([128, S], bf16, tag="kt", name=f"kt{hp}")
                qts[hp], kts[hp] = qt, kt
                for dst, w, b in ((qt, wq, bqs), (kt, wk, bks)):
                    for sc in range(2):
                        sl = slice(sc * 512, (sc + 1) * 512)
                        ps = ps_proj.tile([128, 512], f32, tag="pp",
                                          name=f"pp{hp}_{dst.name}{sc}")
                        for k in range(NE):
                            nc.tensor.matmul(
                                ps,
                                lhsT=w[k][:, hp * 128:(hp + 1) * 128],
                                rhs=xt[k][:, sl],
                                start=(k == 0), stop=(k == NE - 1),
                            )
                        nc.vector.tensor_scalar_add(dst[:, sl], ps, b[hp])

            def emit_v():
                with tc.tile_pool(name="ps_v", bufs=1, space="PSUM") as ps_v:
                    for i in range(NT):
                        ps = ps_v.tile([128, VW], f32, tag="pv", name=f"pv{i}")
                        for k in range(NE + 1):
                            for off, sz in ((0, 512), (512, VW - 512)):
                                nc.tensor.matmul(
                                    ps[:, off:off + sz],
                                    lhsT=xa(k)[:, i * 128:(i + 1) * 128],
                                    rhs=wv[k][:, off:off + sz],
                                    start=(k == 0), stop=(k == NE),
                                )
                        nc.vector.tensor_copy(vt[i], ps)

            emit_qk(0)

            for hp in range(H // 2):
                qt, kt = qts.pop(hp), kts.pop(hp)
                exps = [[], []]
                for t in range(NT):
                    for half in range(2):
                        hb = half * 64
                        ps = ps_sc.tile([128, 1024], f32, tag="sc",
                                        name=f"sc{hp}_{t}_{half}")
                        for sc in range(2):
                            nc.tensor.matmul(
                                ps[:, sc * 512:(sc + 1) * 512],
                                lhsT=kt[hb:hb + 64, t * 128:(t + 1) * 128],
                                rhs=qt[hb:hb + 64, sc * 512:(sc + 1) * 512],
                                start=True, stop=True,
                                tile_position=(hb, 0),
                            )
                        ex = expp.tile([128, 1024], bf16, tag="e",
                                       name=f"e{hp}_{t}_{half}")
                        nc.scalar.activation(ex, ps, EXP, scale=0.125)
                        exps[half].append(ex)
                if hp == 0:
                    emit_v()
                    # V psum banks freed; attention psum takes their place.
                    ps_at = sctx.enter_context(
                        tc.tile_pool(name="ps_at", bufs=1, space="PSUM"))
                if hp + 1 < H // 2:
                    emit_qk(hp + 1)
                last = hp == H // 2 - 1
                asbs, rcbs = [], []
                for half in range(2):
                    head = hp * 2 + half
                    pa = ps_at.tile([HW, 1024], f32, tag="at",
                                    name=f"at{head}")
                    for t in range(NT):
                        for sc in range(2):
                            nc.tensor.matmul(
                                pa[:, sc * 512:(sc + 1) * 512],
                                lhsT=vt[t][:, head * HW:(head + 1) * HW],
                                rhs=exps[half][t][:, sc * 512:(sc + 1) * 512],
                                start=(t == 0), stop=(t == NT - 1),
                            )
                    asb = normp.tile([HW, 1024], f32, tag="asb",
                                     name=f"asb{head}")
                    nc.vector.tensor_copy(asb, pa)
                    # last pair: run the two heads' chains on separate DMA
                    # queues so they pipeline instead of serializing
                    dq = nc.scalar if last and half == 1 else nc.gpsimd
                    dn1 = dscr.tile([1, 1024], f32, tag="d1",
                                    name=f"dn1{head}")
                    dq.dma_start(out=dn1, in_=asb[64:65, :])
                    den8 = normp.tile([128, 8], f32, tag="d8",
                                      name=f"den8{head}")
                    dn1_r = bass.AP(tensor=dn1.tensor, offset=dn1.offset,
                                    ap=[[8, 128], [1, 8]])
                    dq.dma_start(out=den8, in_=dn1_r)
                    rcp8 = normp.tile([128, 8], f32, tag="r8",
                                      name=f"rcp8{head}")
                    nc.vector.reciprocal(rcp8, den8)
                    dn2 = dscr.tile([1, 1024], f32, tag="d2",
                                    name=f"dn2{head}")
                    dn2_w = bass.AP(tensor=dn2.tensor, offset=dn2.offset,
                                    ap=[[8, 128], [1, 8]])
                    dq.dma_start(out=dn2_w, in_=rcp8)
                    rcb = normp.tile([64, 1024], f32, tag="rcb",
                                     name=f"rcb{head}")
                    dq.dma_start(
                        out=rcb, in_=dn2[0].partition_broadcast(64))
                    asbs.append(asb)
                    rcbs.append(rcb)
                    if not last:
                        nc.gpsimd.tensor_mul(
                            catt[hp][half * 64:(half + 1) * 64, :],
                            asb[0:64, :], rcb)
                if last:
                    # muls after both chains are in flight; split by column
                    # halves so the output projection unblocks early
                    for cc in range(2):
                        cs = slice(cc * 512, (cc + 1) * 512)
                        for half in range(2):
                            nc.vector.tensor_mul(
                                catt[hp][half * 64:(half + 1) * 64, cs],
                                asbs[half][0:64, cs], rcbs[half][:, cs])

        # ---- P4: output projection ----
        # Two waves of 4 m-tiles; each psum group is paused after the
        # catt[0..4]+bias part (ready as soon as pair 4 is done, fills the
        # last pair's normalize latency) and finished with the k=5 matmuls
        # once catt[5] lands.  osb tiles live in `singles` so no pool-open
        # alias barrier gates the PE.
        def ca(k):
            return catt[k] if k < NE else ones_row

        osb_t = [singles.tile([128, E], f32, tag=f"osb{i}", name=f"osb{i}")
                 for i in range(4)]
        with tc.tile_pool(name="ps_o", bufs=4, space="PSUM") as ps_o:
            K1 = [0, 1, 2, 3, 4, NE]
            for wave in range(2):
                pss = {}
                for m in range(wave * 4, wave * 4 + 4):
                    ps = ps_o.tile([128, E], f32, tag="po", name=f"po{m}")
                    pss[m] = ps
                    for k in K1:
                        for off, sz in ((0, 512), (512, E - 512)):
                            nc.tensor.matmul(
                                ps[:, off:off + sz],
                                lhsT=ca(k)[:, m * 128:(m + 1) * 128],
                                rhs=wo[k][:, off:off + sz],
                                start=(k == K1[0]), stop=False,
                            )
                for m in range(wave * 4, wave * 4 + 4):
                    ps = pss[m]
                    for off, sz in ((0, 512), (512, E - 512)):
                        nc.tensor.matmul(
                            ps[:, off:off + sz],
                            lhsT=ca(5)[:, m * 128:(m + 1) * 128],
                            rhs=wo[5][:, off:off + sz],
                            start=False, stop=True, skip_group_check=True,
                        )
                    ot = osb_t[m % 4]
                    nc.scalar.copy(ot, ps)
                    nc.sync.dma_start(out=out_d[m * 128:(m + 1) * 128, :],
                                      in_=ot)

    _dedupe_ldweights(nc)
    if split_waits:
        _split_multiwaits(nc)
    return nc


def _tile_img(Wt):
    """[E, width] -> SBUF image [128, NE*width] (row p = k-tile rows p)."""
    width = Wt.shape[1]
    return np.ascontiguousarray(
        Wt.reshape(NE, 128, width).transpose(1, 0, 2).reshape(128, NE * width))


def _prep_weights(Wq, bq, Wk, bk, Wv, bv, Wo, bo):
    bf16 = ml_dtypes.bfloat16

    wqt = _tile_img(np.asarray(Wq, np.float32).reshape(H * DH, E).T).astype(bf16)
    wkt = _tile_img(np.asarray(Wk, np.float32).reshape(H * DH, E).T).astype(bf16)
    bqv = np.ascontiguousarray(
        np.asarray(bq, np.float32).reshape(NE, 128).T).astype(np.float32)
    bkv = np.ascontiguousarray(
        np.asarray(bk, np.float32).reshape(NE, 128).T).astype(np.float32)

    wvt = np.zeros((EA, VW), np.float32)
    Wv = np.asarray(Wv, np.float32)
    bv = np.asarray(bv, np.float32)
    for h in range(H):
        wvt[0:E, h * HW:h * HW + DH] = Wv[h].T
        wvt[E, h * HW:h * HW + DH] = bv[h]
        wvt[E, h * HW + DH] = 1.0
    wvb = wvt[E:EA].astype(bf16)
    wvt = _tile_img(wvt[0:E]).astype(bf16)

    Wo = np.asarray(Wo, np.float32)
    bo = np.asarray(bo, np.float32)
    wot = _tile_img(Wo.T).astype(bf16)
    wob = bo.reshape(1, E).astype(bf16)
    return wqt, wkt, bqv, bkv, wvt, wvb, wot, wob


def _install_ntff_shim():
    """Provide antenv.axon_hooks (absent in this image) so trace=True can
    drive NRT profiling through libaxon_pjrt.so.  Dev-only; harmless no-op
    when anything is missing."""
    import sys, types
    try:
        import antenv.axon_hooks  # noqa
        return
    except ImportError:
        pass
    try:
        import antenv
        mod = types.ModuleType("antenv.axon_hooks")
        _state = {}
        mod.set_axon_ntff_profile_hook = lambda h: _state.update(h=h)
        mod.get_axon_ntff_profile_hook = lambda: _state.get("h")
        sys.modules["antenv.axon_hooks"] = mod
        antenv.axon_hooks = mod
        from trn_agent_boot.trn_boot import _ntff_profile_via_ctypes
        hook = _ntff_profile_via_ctypes("/opt/axon/libaxon_pjrt.so")
        if hook is not None:
            mod.set_axon_ntff_profile_hook(hook)
    except Exception as e:  # pragma: no cover
        print(f"ntff shim failed: {e}")


def kernel(x, Wq, bq, Wk, bk, Wv, bv, Wo, bo):
    from concourse.bass_utils import run_bass_kernel_spmd

    if "nc" not in _cache:
        _cache["nc"] = _build_bass()
    nc = _cache["nc"]

    wqt, wkt, bqv, bkv, wvt, wvb, wot, wob = _prep_weights(
        Wq, bq, Wk, bk, Wv, bv, Wo, bo)
    x = np.asarray(x, np.float32)
    in_maps = [
        {"xtr": _tile_img(np.ascontiguousarray(x[b].T)
                          ).astype(ml_dtypes.bfloat16),
         "wqt": wqt, "wkt": wkt, "bq": bqv, "bk": bkv,
         "wvt": wvt, "wvb": wvb, "wot": wot, "wob": wob}
        for b in range(B)
    ]
    trace = bool(int(os.environ.get("MHA_TRACE", "0")))
    if trace:
        _install_ntff_shim()
    res = run_bass_kernel_spmd(nc, in_maps, list(range(B)), trace=trace)
    _cache["last_results"] = res
    return np.stack([res.results[b]["out"] for b in range(B)]).astype(np.float32)


# revision 33
# speedup vs baseline: 1.0257x; 1.0257x over previous
"""Multi-head attention (B=8, S=1024, E=768, H=12) on 8 trn2 NeuronCores.

Strategy: batch-parallel — core b processes batch element b end-to-end, no
collectives.  Projections/attention/output in bf16 with fp32 PSUM; the
score matmul runs in fp8-e4m3 DoubleRow mode (2 contraction rows per
partition -> full 128-row array use at half the stream cycles) with a
q-side residual (q ~= q8 + qr8) so only the k-side fp8 quantization error
survives (~6e-3 end-to-end, vs 2.4e-3 all-bf16).

Per-core dataflow (token s/t, feature e, head h, head-dim d):
  xT[e, s]    PE-transpose of x in bf16 (cast on ACT first; 2 DMA waves)
  q/k proj    psum[hd, s-chunk] = Wq/Wk-tile^T @ xT; DVE writes
              qf[128, 2, 1024] fp8 (blk0 = fp8(psum+b), blk1 = residual),
              kf same with blk1 = copy of blk0 (SBUF DMA)
  v[t, hdA]   xT_aug^T @ WvT_aug  (ones row gives bias; col h*65+64 is
              all-ones -> softmax denominator), interleaved with pair 0
  scoresT     DoubleRow fp8: lhsT = kf[h*64:, :, t-tile], rhs = qf[h*64:,
              :, s-chunk]; 256 cycles per [128, 512] output
  expT        ACT exp with scale=1/8 (scores kept unscaled), bf16
  attn_aug    v_aug^T @ expT accumulated over t (row 64 = denominator)
  normalize   DVE reciprocal of den row; Pool partition_broadcast to 64
              rows; Pool/DVE multiply -> catT bf16
  out[s, f]   catT_aug^T @ WoT_aug (bias row folded), ACT copy, DMA out
"""

import os
import numpy as np
import ml_dtypes

B, S, E, H, DH = 8, 1024, 768, 12, 64
EA = E + 1          # augmented contraction dim (ones/bias row)
HW = DH + 1         # per-head V width (d cols + ones col)
VW = H * HW         # 780
NT = S // 128       # 8 token tiles
NE = E // 128       # 6 feature tiles

_cache = {}


def _split_multiwaits(nc):
    """This toolchain's walrus encodes at most one sync-wait per instruction
    (two for EventSemaphore).  Tile's epilogue can attach more; hoist the
    extras onto same-engine NOPs placed immediately before the instruction —
    the engine sequencer executes in order, so semantics are unchanged."""
    import concourse.mybir as mybir

    for bb in nc.main_func.blocks:
        out, changed = [], False
        for ins in bb.instructions:
            si = ins.sync_info
            cap = 2 if isinstance(ins, mybir.InstEventSemaphore) else 1
            if si is not None and si.on_wait and len(si.on_wait) > cap:
                waits = list(si.on_wait)
                for w_i, w in enumerate(waits[:-cap]):
                    out.append(mybir.InstNoOp(
                        name=f"{ins.name}-wsplit{w_i}",
                        engine=ins.engine,
                        sync_info=mybir.SyncInfo(on_wait=[w], on_update=[]),
                        bass_nofuse=True,
                    ))
                ins.sync_info = mybir.SyncInfo(
                    on_wait=waits[-cap:], on_update=list(si.on_update))
                changed = True
            out.append(ins)
        if changed:
            bb.instructions = out


def _dedupe_ldweights(nc):
    """Delete an InstLdweights when the immediately-preceding PE-stream
    instructions are its identical twin followed only by plain (non-transpose)
    matmuls — the weights are still resident in the array.  Only waitless,
    updateless LDWs are removed."""
    import concourse.mybir as mybir

    ndel = 0
    for bb in nc.main_func.blocks:
        out = []
        prev_key = None          # signature of weights currently in the array
        changed = False
        for ins in bb.instructions:
            if isinstance(ins, mybir.InstLdweights):
                si = ins.sync_info
                clean = not si or (not si.on_wait and not si.on_update)
                key = (str(ins.ins[0]), str(ins.tile_position),
                       str(ins.perf_mode), str(ins.is_transpose))
                if clean and key == prev_key:
                    ndel += 1
                    changed = True
                    continue
                prev_key = key
            elif isinstance(ins, mybir.InstMatmult):
                if ins.is_transpose:
                    prev_key = None   # transpose streams data into the array
            elif ins.engine == mybir.EngineType.PE:
                prev_key = None
            out.append(ins)
        if changed:
            bb.instructions = out
    return ndel


def _build_bass(split_waits=True):
    import concourse.bass as bass
    import concourse.tile as tile
    import concourse.mybir as mybir
    from contextlib import ExitStack

    f32 = mybir.dt.float32
    bf16 = mybir.dt.bfloat16
    EXP = mybir.ActivationFunctionType.Exp

    nc = bass.Bass(trn_type="TRN2")

    # All inputs arrive host-pre-tiled as the exact SBUF image
    # [128, NE*width] so every load is 128 contiguous descriptors.
    xt_d = nc.dram_tensor("xtr", [128, NE * S], bf16, kind="ExternalInput")
    wqt_d = nc.dram_tensor("wqt", [128, NE * E], bf16, kind="ExternalInput")
    wkt_d = nc.dram_tensor("wkt", [128, NE * E], bf16, kind="ExternalInput")
    bq_d = nc.dram_tensor("bq", [128, NE], f32, kind="ExternalInput")
    bk_d = nc.dram_tensor("bk", [128, NE], f32, kind="ExternalInput")
    wvt_d = nc.dram_tensor("wvt", [128, NE * VW], bf16, kind="ExternalInput")
    wvb_d = nc.dram_tensor("wvb", [1, VW], bf16, kind="ExternalInput")
    wot_d = nc.dram_tensor("wot", [128, NE * E], bf16, kind="ExternalInput")
    wob_d = nc.dram_tensor("wob", [1, E], bf16, kind="ExternalInput")
    out_d = nc.dram_tensor("out", [S, E], f32, kind="ExternalOutput")

    with tile.TileContext(nc) as tc, ExitStack() as ctx:
        singles = ctx.enter_context(tc.tile_pool(name="singles", bufs=1))

        ones_row = singles.tile([1, 1024], bf16)
        nc.vector.memset(ones_row, 1.0)

        # ---- input DMAs.  Big transfers serialize on the SP queue in
        # priority order (descriptor-gen of DMA n+1 overlaps transfer n);
        # the two tiny bias loads go to the Pool queue. ----
        xt_all = singles.tile([128, NE * S], bf16, tag="xta", name="xtall")
        nc.sync.dma_start(out=xt_all, in_=xt_d[:, :])
        xt = [xt_all[:, j * S:(j + 1) * S] for j in range(NE)]

        # biases as one [128, NE] tile each (column m = k-tile m)
        bq_sb = singles.tile([128, NE], f32, tag="bqs", name="bqs")
        nc.gpsimd.dma_start(out=bq_sb, in_=bq_d[:, :])
        bk_sb = singles.tile([128, NE], f32, tag="bks", name="bks")
        nc.gpsimd.dma_start(out=bk_sb, in_=bk_d[:, :])
        bqs = [bq_sb[:, m:m + 1] for m in range(NE)]
        bks = [bk_sb[:, m:m + 1] for m in range(NE)]

        class WView:
            """All k-tiles of a weight in one SBUF tile (one DMA)."""
            def __init__(self, all_tile, width, bias_tile):
                self.all, self.width, self.bias = all_tile, width, bias_tile

            def __getitem__(self, k):
                if self.bias is not None and k == NE:
                    return self.bias
                return _WSlice(self, k)

        class _WSlice:
            def __init__(self, v, k):
                self.v, self.k = v, k

            def __getitem__(self, idx):
                _, cols = idx
                off = self.k * self.v.width
                return self.v.all[:, off + cols.start:off + cols.stop]

        def load_w(dram, width, bias_dram):
            t = singles.tile([128, NE * width], bf16, tag=f"w{dram.name}",
                             name=f"w{dram.name}")
            nc.sync.dma_start(out=t, in_=dram[:, :])
            bias_t = None
            if bias_dram is not None:
                bias_t = singles.tile([1, width], bf16, tag=f"w{dram.name}b",
                                      name=f"w{dram.name}b")
                nc.gpsimd.dma_start(out=bias_t, in_=bias_dram[:, :])
            return WView(t, width, bias_t)

        wq = load_w(wqt_d, E, None)
        wk = load_w(wkt_d, E, None)
        wv = load_w(wvt_d, VW, wvb_d)
        wo = load_w(wot_d, E, wob_d)

        def xa(k):  # augmented xT rows
            return xt[k] if k < NE else ones_row

        # ---- steady-state tiles ----
        vt = [singles.tile([128, VW], bf16, tag=f"vt{i}", name=f"vt{i}")
              for i in range(NT)]
        catt = [singles.tile([128, S], bf16, tag=f"ct{j}", name=f"ct{j}")
                for j in range(NE)]

        with ExitStack() as sctx:
            qk8p = sctx.enter_context(tc.tile_pool(name="qk8", bufs=2))
            expp = sctx.enter_context(tc.tile_pool(name="exp", bufs=18))
            normp = sctx.enter_context(tc.tile_pool(name="norm", bufs=2))
            ps_proj = sctx.enter_context(
                tc.tile_pool(name="ps_proj", bufs=2, space="PSUM"))
            ps_sc = sctx.enter_context(
                tc.tile_pool(name="ps_sc", bufs=2, space="PSUM"))
            dscr = sctx.enter_context(
                tc.tile_pool(name="dscr", bufs=8, space="DRAM"))

            qts, kts = {}, {}

            def emit_qk(hp):
                qt = qk8p.tile([128, S], bf16, tag="qt", name=f"qt{hp}")
                kt = qk8p.tile([128, S], bf16, tag="kt", name=f"kt{hp}")
                qts[hp], kts[hp] = qt, kt
                for dst, w, b in ((qt, wq, bqs), (kt, wk, bks)):
                    for sc in range(2):
                        sl = slice(sc * 512, (sc + 1) * 512)
                        ps = ps_proj.tile([128, 512], f32, tag="pp",
                                          name=f"pp{hp}_{dst.name}{sc}")
                        for k in range(NE):
                            nc.tensor.matmul(
                                ps,
                                lhsT=w[k][:, hp * 128:(hp + 1) * 128],
                                rhs=xt[k][:, sl],
                                start=(k == 0), stop=(k == NE - 1),
                            )
                        nc.vector.tensor_scalar_add(dst[:, sl], ps, b[hp])

            def emit_v():
                with tc.tile_pool(name="ps_v", bufs=1, space="PSUM") as ps_v:
                    for i in range(NT):
                        ps = ps_v.tile([128, VW], f32, tag="pv", name=f"pv{i}")
                        for k in range(NE + 1):
                            for off, sz in ((0, 512), (512, VW - 512)):
                                nc.tensor.matmul(
                                    ps[:, off:off + sz],
                                    lhsT=xa(k)[:, i * 128:(i + 1) * 128],
                                    rhs=wv[k][:, off:off + sz],
                                    start=(k == 0), stop=(k == NE),
                                )
                        nc.vector.tensor_copy(vt[i], ps)

            emit_qk(0)

            for hp in range(H // 2):
                qt, kt = qts.pop(hp), kts.pop(hp)
                exps = [[], []]
                for t in range(NT):
                    for half in range(2):
                        hb = half * 64
                        ps = ps_sc.tile([128, 1024], f32, tag="sc",
                                        name=f"sc{hp}_{t}_{half}")
                        for sc in range(2):
                            nc.tensor.matmul(
                                ps[:, sc * 512:(sc + 1) * 512],
                                lhsT=kt[hb:hb + 64, t * 128:(t + 1) * 128],
                                rhs=qt[hb:hb + 64, sc * 512:(sc + 1) * 512],
                                start=True, stop=True,
                                tile_position=(hb, 0),
                            )
                        ex = expp.tile([128, 1024], bf16, tag="e",
                                       name=f"e{hp}_{t}_{half}")
                        nc.scalar.activation(ex, ps, EXP, scale=0.125)
                        exps[half].append(ex)
                if hp == 0:
                    emit_v()
                    # V psum banks freed; attention psum takes their place.
                    ps_at = sctx.enter_context(
                        tc.tile_pool(name="ps_at", bufs=1, space="PSUM"))
                if hp + 1 < H // 2:
                    emit_qk(hp + 1)
                last = hp == H // 2 - 1
                asbs, rcbs = [], []
                for half in range(2):
                    head = hp * 2 + half
                    pa = ps_at.tile([HW, 1024], f32, tag="at",
                                    name=f"at{head}")
                    for t in range(NT):
                        for sc in range(2):
                            nc.tensor.matmul(
                                pa[:, sc * 512:(sc + 1) * 512],
                                lhsT=vt[t][:, head * HW:(head + 1) * HW],
                                rhs=exps[half][t][:, sc * 512:(sc + 1) * 512],
                                start=(t == 0), stop=(t == NT - 1),
                            )
                    asb = normp.tile([HW, 1024], f32, tag="asb",
                                     name=f"asb{head}")
                    nc.vector.tensor_copy(asb, pa)
                    # last pair: run the two heads' chains on separate DMA
                    # queues so they pipeline instead of serializing
                    dq = nc.scalar if last and half == 1 else nc.gpsimd
                    dn1 = dscr.tile([1, 1024], f32, tag="d1",
                                    name=f"dn1{head}")
                    dq.dma_start(out=dn1, in_=asb[64:65, :])
                    den8 = normp.tile([128, 8], f32, tag="d8",
                                      name=f"den8{head}")
                    dn1_r = bass.AP(tensor=dn1.tensor, offset=dn1.offset,
                                    ap=[[8, 128], [1, 8]])
                    dq.dma_start(out=den8, in_=dn1_r)
                    rcp8 = normp.tile([128, 8], f32, tag="r8",
                                      name=f"rcp8{head}")
                    nc.vector.reciprocal(rcp8, den8)
                    dn2 = dscr.tile([1, 1024], f32, tag="d2",
                                    name=f"dn2{head}")
                    dn2_w = bass.AP(tensor=dn2.tensor, offset=dn2.offset,
                                    ap=[[8, 128], [1, 8]])
                    dq.dma_start(out=dn2_w, in_=rcp8)
                    rcb = normp.tile([64, 1024], f32, tag="rcb",
                                     name=f"rcb{head}")
                    dq.dma_start(
                        out=rcb, in_=dn2[0].partition_broadcast(64))
                    asbs.append(asb)
                    rcbs.append(rcb)
                    if not last:
                        nc.gpsimd.tensor_mul(
                            catt[hp][half * 64:(half + 1) * 64, :],
                            asb[0:64, :], rcb)
                if last:
                    # muls after both chains are in flight; split by column
                    # halves so the output projection unblocks early
                    for cc in range(2):
                        cs = slice(cc * 512, (cc + 1) * 512)
                        for half in range(2):
                            nc.vector.tensor_mul(
                                catt[hp][half * 64:(half + 1) * 64, cs],
                                asbs[half][0:64, cs], rcbs[half][:, cs])

        # ---- P4: output projection ----
        # Two waves of 4 m-tiles; each psum group is paused after the
        # catt[0..4]+bias part (ready as soon as pair 4 is done, fills the
        # last pair's normalize latency) and finished with the k=5 matmuls
        # once catt[5] lands.  osb tiles live in `singles` so no pool-open
        # alias barrier gates the PE.
        def ca(k):
            return catt[k] if k < NE else ones_row

        osb_t = [singles.tile([128, E], f32, tag=f"osb{i}", name=f"osb{i}")
                 for i in range(4)]
        with tc.tile_pool(name="ps_o", bufs=4, space="PSUM") as ps_o:
            KL = [0, 1, 2, 3, 4, NE, 5]
            for m in range(NT):
                ps = ps_o.tile([128, E], f32, tag="po", name=f"po{m}")
                for k in KL:
                    for off, sz in ((0, 512), (512, E - 512)):
                        nc.tensor.matmul(
                            ps[:, off:off + sz],
                            lhsT=ca(k)[:, m * 128:(m + 1) * 128],
                            rhs=wo[k][:, off:off + sz],
                            start=(k == KL[0]), stop=(k == KL[-1]),
                        )
                ot = osb_t[m % 4]
                nc.scalar.copy(ot, ps)
                nc.sync.dma_start(out=out_d[m * 128:(m + 1) * 128, :],
                                  in_=ot)

    _dedupe_ldweights(nc)
    if split_waits:
        _split_multiwaits(nc)
    return nc


def _tile_img(Wt):
    """[E, width] -> SBUF image [128, NE*width] (row p = k-tile rows p)."""
    width = Wt.shape[1]
    return np.ascontiguousarray(
        Wt.reshape(NE, 128, width).transpose(1, 0, 2).reshape(128, NE * width))


def _prep_weights(Wq, bq, Wk, bk, Wv, bv, Wo, bo):
    bf16 = ml_dtypes.bfloat16

    wqt = _tile_img(np.asarray(Wq, np.float32).reshape(H * DH, E).T).astype(bf16)
    wkt = _tile_img(np.asarray(Wk, np.float32).reshape(H * DH, E).T).astype(bf16)
    bqv = np.ascontiguousarray(
        np.asarray(bq, np.float32).reshape(NE, 128).T).astype(np.float32)
    bkv = np.ascontiguousarray(
        np.asarray(bk, np.float32).reshape(NE, 128).T).astype(np.float32)

    wvt = np.zeros((EA, VW), np.float32)
    Wv = np.asarray(Wv, np.float32)
    bv = np.asarray(bv, np.float32)
    for h in range(H):
        wvt[0:E, h * HW:h * HW + DH] = Wv[h].T
        wvt[E, h * HW:h * HW + DH] = bv[h]
        wvt[E, h * HW + DH] = 1.0
    wvb = wvt[E:EA].astype(bf16)
    wvt = _tile_img(wvt[0:E]).astype(bf16)

    Wo = np.asarray(Wo, np.float32)
    bo = np.asarray(bo, np.float32)
    wot = _tile_img(Wo.T).astype(bf16)
    wob = bo.reshape(1, E).astype(bf16)
    return wqt, wkt, bqv, bkv, wvt, wvb, wot, wob


def _install_ntff_shim():
    """Provide antenv.axon_hooks (absent in this image) so trace=True can
    drive NRT profiling through libaxon_pjrt.so.  Dev-only; harmless no-op
    when anything is missing."""
    import sys, types
    try:
        import antenv.axon_hooks  # noqa
        return
    except ImportError:
        pass
    try:
        import antenv
        mod = types.ModuleType("antenv.axon_hooks")
        _state = {}
        mod.set_axon_ntff_profile_hook = lambda h: _state.update(h=h)
        mod.get_axon_ntff_profile_hook = lambda: _state.get("h")
        sys.modules["antenv.axon_hooks"] = mod
        antenv.axon_hooks = mod
        from trn_agent_boot.trn_boot import _ntff_profile_via_ctypes
        hook = _ntff_profile_via_ctypes("/opt/axon/libaxon_pjrt.so")
        if hook is not None:
            mod.set_axon_ntff_profile_hook(hook)
    except Exception as e:  # pragma: no cover
        print(f"ntff shim failed: {e}")


def kernel(x, Wq, bq, Wk, bk, Wv, bv, Wo, bo):
    from concourse.bass_utils import run_bass_kernel_spmd

    if "nc" not in _cache:
        _cache["nc"] = _build_bass()
    nc = _cache["nc"]

    wqt, wkt, bqv, bkv, wvt, wvb, wot, wob = _prep_weights(
        Wq, bq, Wk, bk, Wv, bv, Wo, bo)
    x = np.asarray(x, np.float32)
    in_maps = [
        {"xtr": _tile_img(np.ascontiguousarray(x[b].T)
                          ).astype(ml_dtypes.bfloat16),
         "wqt": wqt, "wkt": wkt, "bq": bqv, "bk": bkv,
         "wvt": wvt, "wvb": wvb, "wot": wot, "wob": wob}
        for b in range(B)
    ]
    trace = bool(int(os.environ.get("MHA_TRACE", "0")))
    if trace:
        _install_ntff_shim()
    res = run_bass_kernel_spmd(nc, in_maps, list(range(B)), trace=trace)
    _cache["last_results"] = res
    return np.stack([res.results[b]["out"] for b in range(B)]).astype(np.float32)
